# revision 1
# baseline (speedup 1.0000x reference)
import sys

sys.path.insert(0, "/opt/trn_rl_repo")
import numpy as np

B, S, D, H, R = 2, 2048, 768, 12, 16
LORA_SCALE = 1.0 / R
W = D // H  # 64
HPC = 3  # heads per core
WPC = HPC * W  # 192 output dims per core
NCORES = 8
SB = 512  # s-block for projections
NT = S // 128  # 16 t-chunks

_cache = {}


def _build():
    import concourse.bacc as bacc
    import concourse.mybir as mybir
    import concourse.tile as tile

    f32 = mybir.dt.float32
    bf16 = mybir.dt.bfloat16
    AF = mybir.ActivationFunctionType

    nc = bacc.Bacc("TRN2", target_bir_lowering=False, debug=False)
    xT_d = nc.dram_tensor("xT", [D, S], bf16, kind="ExternalInput")
    WAT_d = nc.dram_tensor("WAT", [D, 432], bf16, kind="ExternalInput")
    WvT_d = nc.dram_tensor("WvT", [D, WPC], bf16, kind="ExternalInput")
    BqT_d = nc.dram_tensor("BqT", [R, WPC], bf16, kind="ExternalInput")
    BvT_d = nc.dram_tensor("BvT", [R, WPC], bf16, kind="ExternalInput")
    bias_d = nc.dram_tensor("bias_qk", [128, 4], f32, kind="ExternalInput")
    bv_d = nc.dram_tensor("bv_row", [1, WPC], bf16, kind="ExternalInput")
    mb_d = nc.dram_tensor("mb", [128, NT], f32, kind="ExternalInput")
    out_d = nc.dram_tensor("outT", [HPC * 65, S], f32, kind="ExternalOutput")

    with tile.TileContext(nc) as tc:
        with tc.tile_pool(name="cst", bufs=1) as cst:
            xT = cst.tile([128, 6, S], bf16, name="xT")
            WAT = cst.tile([128, 6, 432], bf16, name="WAT")
            WvT = cst.tile([128, 6, WPC], bf16, name="WvT")
            BqT = cst.tile([R, WPC], bf16, name="BqT")
            BvT = cst.tile([49, WPC], bf16, name="BvT")
            bias = cst.tile([128, 4], f32, name="bias")
            mb = cst.tile([128, NT], f32, name="mb")
            QT = cst.tile([128, 2, S], bf16, name="QT")
            KT = cst.tile([128, 2, S], bf16, name="KT")
            u = cst.tile([49, S], bf16, name="u")  # 0:16 uq, 32:48 uv, 48 ones
            V = cst.tile([128, NT, 195], bf16, name="V")
            OT = [cst.tile([65, S], f32, name=f"ot{h}") for h in range(HPC)]

            nc.sync.dma_start(xT[:], xT_d.ap().rearrange("(c p) s -> p c s", p=128))
            nc.sync.dma_start(WAT[:], WAT_d.ap().rearrange("(c p) m -> p c m", p=128))
            nc.sync.dma_start(WvT[:], WvT_d.ap().rearrange("(c p) m -> p c m", p=128))
            nc.gpsimd.dma_start(BqT[:], BqT_d.ap())
            nc.gpsimd.dma_start(BvT[32:48, :], BvT_d.ap())
            nc.gpsimd.dma_start(bias[:], bias_d.ap())
            nc.gpsimd.dma_start(BvT[48:49, :], bv_d.ap())
            nc.vector.memset(u[32:49, :], 1.0)
            nc.gpsimd.dma_start(mb[:], mb_d.ap())
            nc.vector.memset(V[:, :, 64::65], 1.0)
            tc.strict_bb_all_engine_barrier()

            # ---- phase 1: projections ----
            import os
            LVL = int(os.environ.get("P1LVL", "9"))
            # W_A cols: q 0:192 | k 192:384 | Aq 384:400 | pad | Av 416:432
            chunk_cols = [(0, 128), (128, 192), (192, 320), (320, 384)]
            drains = [
                (QT, 0, 128, 0), (QT, 1, 64, 1), (KT, 0, 128, 2), (KT, 1, 64, 3),
            ]
            with (
                tc.tile_pool(name="pu0", bufs=1, space="PSUM") as pu_pool,
                tc.tile_pool(name="pc0", bufs=1, space="PSUM") as pc0,
                tc.tile_pool(name="pc1", bufs=1, space="PSUM") as pc1,
                tc.tile_pool(name="pc2", bufs=1, space="PSUM") as pc2,
                tc.tile_pool(name="pc3", bufs=1, space="PSUM") as pc3,
                tc.tile_pool(name="vpa", bufs=1, space="PSUM") as vpa,
                tc.tile_pool(name="vpb", bufs=1, space="PSUM") as vpb,
            ):
                pc = [pc0, pc1, pc2, pc3]
                for sb in range(S // SB if LVL >= 1 else 0):
                    ssl = slice(sb * SB, (sb + 1) * SB)
                    pu = pu_pool.tile([48, SB], f32, name="pu")
                    for c in range(6):
                        nc.tensor.matmul(
                            pu[:], WAT[:, c, 384:432], xT[:, c, ssl],
                            start=(c == 0), stop=(c == 5),
                        )
                    nc.vector.tensor_copy(u[0:48, ssl], pu[:])
                    for ci in range(4 if LVL >= 2 else 0):
                        c0, c1 = chunk_cols[ci]
                        m = c1 - c0
                        p = pc[ci].tile([128, SB], f32, name=f"pc{ci}t")
                        has_lora = ci < 2
                        if has_lora:
                            nc.tensor.matmul(
                                p[:m], BqT[:, c0:c1], u[0:16, ssl],
                                start=True, stop=False, skip_group_check=True,
                            )
                        for c in range(6):
                            nc.tensor.matmul(
                                p[:m], WAT[:, c, c0:c1], xT[:, c, ssl],
                                start=(c == 0 and not has_lora), stop=(c == 5),
                                skip_group_check=True,
                            )
                        dst, di, dm, bc = drains[ci]
                        nc.vector.tensor_scalar_add(
                            dst[0:dm, di, ssl], p[0:dm], bias[0:dm, bc:bc + 1]
                        )

                # V: normal layout [s, w] per 128-chunk
                for t in range(NT if LVL >= 3 else 0):
                    tsl = slice(t * 128, (t + 1) * 128)
                    p = (vpa if t % 2 == 0 else vpb).tile([128, WPC], f32, name="vpt")
                    nc.tensor.matmul(p[:], u[32:49, tsl], BvT[32:49, :], start=True,
                                     stop=False, skip_group_check=True)
                    for c in range(6):
                        nc.tensor.matmul(
                            p[:], xT[:, c, tsl], WvT[:, c, :],
                            start=False, stop=(c == 5), skip_group_check=True,
                        )
                    for hh in range(HPC):
                        nc.vector.tensor_copy(V[:, t, hh * 65:hh * 65 + 64],
                                              p[:, hh * 64:(hh + 1) * 64])

            # ---- phase 2: attention ----
            import os
            if os.environ.get("PHASE1_ONLY"):
                for h in range(HPC):
                    nc.gpsimd.dma_start(out_d.ap()[h * 65:(h + 1) * 65, :], QT[0:65, 0, :])
                phase2 = False
            else:
                phase2 = True
            qk_src = [(QT, 0, 0), (QT, 0, 64), (QT, 1, 0)]
            with (
                tc.tile_pool(name="sp", bufs=1, space="PSUM") as sp,
                tc.tile_pool(name="op", bufs=1, space="PSUM") as op,
                tc.tile_pool(name="pt", bufs=2) as ptp,
            ):
                for h in range(HPC if phase2 else 0):
                    _, ci, pb = qk_src[h]
                    q_ap = QT[pb:pb + 64, ci, :]
                    outp = op.tile([65, S], f32, name="op")
                    for t in range(NT):
                        spt = sp.tile([128, S], f32, name="sp")
                        for nb in range(S // 512):
                            nsl = slice(nb * 512, (nb + 1) * 512)
                            nc.tensor.matmul(
                                spt[:, nsl], KT[pb:pb + 64, ci, t * 128:(t + 1) * 128],
                                q_ap[:, nsl], start=True, stop=True,
                            )
                        ptt = ptp.tile([128, S], bf16, name="pt")
                        for hf in range(2):
                            hsl = slice(hf * 1024, (hf + 1) * 1024)
                            nc.scalar.activation(
                                ptt[:, hsl], spt[:, hsl], AF.Exp,
                                bias=mb[:, t:t + 1], scale=1.0,
                            )
                        for nb in range(S // 512):
                            nsl = slice(nb * 512, (nb + 1) * 512)
                            nc.tensor.matmul(
                                outp[:, nsl], V[:, t, h * 65:h * 65 + 65],
                                ptt[:, nsl], start=(t == 0), stop=(t == NT - 1),
                                skip_group_check=True,
                            )
                    nc.scalar.activation(OT[h][:], outp[:], AF.Copy, bias=0.0)
                    nc.sync.dma_start(out_d.ap()[h * 65:(h + 1) * 65, :], OT[h][:])

    nc.compile()
    return nc


def kernel(x, mask, Wq, bq, Aq, Bq, Wk, bk, Wv, bv, Av, Bv):
    from concourse import bass_utils

    x, mask = np.asarray(x), np.asarray(mask)
    Wq, bq, Aq, Bq = map(np.asarray, (Wq, bq, Aq, Bq))
    Wk, bk, Wv, bv, Av, Bv = map(np.asarray, (Wk, bk, Wv, bv, Av, Bv))
    isc = 1.0 / np.sqrt(np.float32(W))

    in_maps = []
    for core in range(NCORES):
        b, g = core // 4, core % 4
        rows = slice(g * WPC, (g + 1) * WPC)
        Wq_s = (Wq[rows] * isc).astype(np.float32)
        bq_s = (bq[rows] * isc).astype(np.float32)
        Bq_s = (Bq[rows] * (isc * LORA_SCALE)).astype(np.float32)
        Wk_s, bk_s = Wk[rows], bk[rows]
        Wv_s, bv_s = Wv[rows], bv[rows]
        Bv_s = (Bv[rows] * LORA_SCALE).astype(np.float32)
        WA = np.concatenate(
            [Wq_s, Wk_s, Aq, np.zeros((16, D), np.float32), Av], axis=0
        )  # [432, 768]
        bias = np.zeros((128, 4), np.float32)
        bias[:, 0] = bq_s[0:128]
        bias[0:64, 1] = bq_s[128:192]
        bias[:, 2] = bk_s[0:128]
        bias[0:64, 3] = bk_s[128:192]
        mb = (-10000.0 * (1.0 - mask[b].astype(np.float32))).reshape(NT, 128).T
        in_maps.append({
            "xT": _bf(np.ascontiguousarray(x[b].T)),
            "WAT": _bf(np.ascontiguousarray(WA.T)),
            "WvT": _bf(np.ascontiguousarray(Wv_s.T)),
            "BqT": _bf(np.ascontiguousarray(Bq_s.T)),
            "BvT": _bf(np.ascontiguousarray(Bv_s.T)),
            "bias_qk": bias,
            "bv_row": _bf(bv_s.reshape(1, WPC)),
            "mb": np.ascontiguousarray(mb),
            }
        )

    global _last_in_maps
    _last_in_maps = in_maps
    if "nc" not in _cache:
        _cache["nc"] = _build()
    res = bass_utils.run_bass_kernel_spmd(
        _cache["nc"], in_maps, core_ids=list(range(NCORES))
    )
    out = np.empty((B, S, D), np.float32)
    for core in range(NCORES):
        b, g = core // 4, core % 4
        ot = res.results[core]["outT"].reshape(HPC, 65, S)
        for h in range(HPC):
            blk = ot[h, 0:64, :] / ot[h, 64:65, :]
            out[b, :, g * WPC + h * W:(g * WPC) + (h + 1) * W] = blk.T
    return out


def _bf(a):
    import jax.numpy as jnp

    return np.asarray(jnp.asarray(np.asarray(a, np.float32), jnp.bfloat16))



# revision 2
# speedup vs baseline: 1378.8351x; 1378.8351x over previous
import sys

sys.path.insert(0, "/opt/trn_rl_repo")
import numpy as np

B, S, D, H, R = 2, 2048, 768, 12, 16
LORA_SCALE = 1.0 / R
W = D // H  # 64
HPC = 3  # heads per core
WPC = HPC * W  # 192 output dims per core
NCORES = 8
SB = 512  # s-block for projections
NT = S // 128  # 16 t-chunks

_cache = {}


def _build():
    import concourse.bacc as bacc
    import concourse.mybir as mybir
    import concourse.tile as tile

    f32 = mybir.dt.float32
    bf16 = mybir.dt.bfloat16
    AF = mybir.ActivationFunctionType

    nc = bacc.Bacc("TRN2", target_bir_lowering=False, debug=False)
    xT_d = nc.dram_tensor("xT", [D, S], bf16, kind="ExternalInput")
    WAT_d = nc.dram_tensor("WAT", [D, 432], bf16, kind="ExternalInput")
    WvT_d = nc.dram_tensor("WvT", [D, WPC], bf16, kind="ExternalInput")
    BqT_d = nc.dram_tensor("BqT", [R, WPC], bf16, kind="ExternalInput")
    BvT_d = nc.dram_tensor("BvT", [R, WPC], bf16, kind="ExternalInput")
    bias_d = nc.dram_tensor("bias_qk", [128, 4], f32, kind="ExternalInput")
    bv_d = nc.dram_tensor("bv_row", [1, WPC], bf16, kind="ExternalInput")
    mb_d = nc.dram_tensor("mb", [128, NT], f32, kind="ExternalInput")
    out_d = nc.dram_tensor("outT", [HPC * 65, S], f32, kind="ExternalOutput")

    with tile.TileContext(nc) as tc:
        with tc.tile_pool(name="cst", bufs=1) as cst:
            xT = cst.tile([128, 6, S], bf16, name="xT")
            WAT = cst.tile([128, 6, 432], bf16, name="WAT")
            WvT = cst.tile([128, 6, WPC], bf16, name="WvT")
            BqT = cst.tile([R, WPC], bf16, name="BqT")
            BvT = cst.tile([49, WPC], bf16, name="BvT")
            bias = cst.tile([128, 4], f32, name="bias")
            mb = cst.tile([128, NT], f32, name="mb")
            QT = cst.tile([128, 2, S], bf16, name="QT")
            KT = cst.tile([128, 2, S], bf16, name="KT")
            u = cst.tile([49, S], bf16, name="u")  # 0:16 uq, 32:48 uv, 48 ones
            V = cst.tile([128, NT, 195], bf16, name="V")
            OT = [cst.tile([65, S], f32, name=f"ot{h}") for h in range(HPC)]

            nc.sync.dma_start(xT[:], xT_d.ap().rearrange("(c p) s -> p c s", p=128))
            nc.sync.dma_start(WAT[:], WAT_d.ap().rearrange("(c p) m -> p c m", p=128))
            nc.sync.dma_start(WvT[:], WvT_d.ap().rearrange("(c p) m -> p c m", p=128))
            nc.gpsimd.dma_start(BqT[:], BqT_d.ap())
            nc.gpsimd.dma_start(BvT[32:48, :], BvT_d.ap())
            nc.gpsimd.dma_start(bias[:], bias_d.ap())
            nc.gpsimd.dma_start(BvT[48:49, :], bv_d.ap())
            nc.vector.memset(u[32:49, :], 1.0)
            nc.gpsimd.dma_start(mb[:], mb_d.ap())
            nc.vector.memset(V[:, :, 64::65], 1.0)
            tc.strict_bb_all_engine_barrier()

            # ---- phase 1: projections ----
            # W_A cols: q 0:192 | k 192:384 | Aq 384:400 | pad | Av 416:432
            chunk_cols = [(0, 128), (128, 192), (192, 320), (320, 384)]
            drains = [
                (QT, 0, 128, 0), (QT, 1, 64, 1), (KT, 0, 128, 2), (KT, 1, 64, 3),
            ]
            with (
                tc.tile_pool(name="pu0", bufs=1, space="PSUM") as pu_pool,
                tc.tile_pool(name="pc0", bufs=1, space="PSUM") as pc0,
                tc.tile_pool(name="pc1", bufs=1, space="PSUM") as pc1,
                tc.tile_pool(name="pc2", bufs=1, space="PSUM") as pc2,
                tc.tile_pool(name="pc3", bufs=1, space="PSUM") as pc3,
                tc.tile_pool(name="vpa", bufs=1, space="PSUM") as vpa,
                tc.tile_pool(name="vpb", bufs=1, space="PSUM") as vpb,
            ):
                pc = [pc0, pc1, pc2, pc3]
                for sb in range(S // SB):
                    ssl = slice(sb * SB, (sb + 1) * SB)
                    pu = pu_pool.tile([48, SB], f32, name="pu")
                    for c in range(6):
                        nc.tensor.matmul(
                            pu[:], WAT[:, c, 384:432], xT[:, c, ssl],
                            start=(c == 0), stop=(c == 5),
                        )
                    nc.vector.tensor_copy(u[0:48, ssl], pu[:])
                    for ci in range(4):
                        c0, c1 = chunk_cols[ci]
                        m = c1 - c0
                        p = pc[ci].tile([128, SB], f32, name=f"pc{ci}t")
                        has_lora = ci < 2
                        if has_lora:
                            nc.tensor.matmul(
                                p[:m], BqT[:, c0:c1], u[0:16, ssl],
                                start=True, stop=False, skip_group_check=True,
                            )
                        for c in range(6):
                            nc.tensor.matmul(
                                p[:m], WAT[:, c, c0:c1], xT[:, c, ssl],
                                start=(c == 0 and not has_lora), stop=(c == 5),
                                skip_group_check=True,
                            )
                        dst, di, dm, bc = drains[ci]
                        nc.vector.tensor_scalar_add(
                            dst[0:dm, di, ssl], p[0:dm], bias[0:dm, bc:bc + 1]
                        )

                # V: normal layout [s, w] per 128-chunk
                for t in range(NT):
                    tsl = slice(t * 128, (t + 1) * 128)
                    p = (vpa if t % 2 == 0 else vpb).tile([128, WPC], f32, name="vpt")
                    nc.tensor.matmul(p[:], u[32:49, tsl], BvT[32:49, :], start=True,
                                     stop=False, skip_group_check=True)
                    for c in range(6):
                        nc.tensor.matmul(
                            p[:], xT[:, c, tsl], WvT[:, c, :],
                            start=False, stop=(c == 5), skip_group_check=True,
                        )
                    for hh in range(HPC):
                        nc.vector.tensor_copy(V[:, t, hh * 65:hh * 65 + 64],
                                              p[:, hh * 64:(hh + 1) * 64])

            # ---- phase 2: attention ----
            qk_src = [(QT, 0, 0), (QT, 0, 64), (QT, 1, 0)]
            with (
                tc.tile_pool(name="sp", bufs=1, space="PSUM") as sp,
                tc.tile_pool(name="op", bufs=1, space="PSUM") as op,
                tc.tile_pool(name="pt", bufs=2) as ptp,
            ):
                for h in range(HPC):
                    _, ci, pb = qk_src[h]
                    q_ap = QT[pb:pb + 64, ci, :]
                    outp = op.tile([65, S], f32, name="op")
                    for t in range(NT):
                        spt = sp.tile([128, S], f32, name="sp")
                        for nb in range(S // 512):
                            nsl = slice(nb * 512, (nb + 1) * 512)
                            nc.tensor.matmul(
                                spt[:, nsl], KT[pb:pb + 64, ci, t * 128:(t + 1) * 128],
                                q_ap[:, nsl], start=True, stop=True,
                            )
                        ptt = ptp.tile([128, S], bf16, name="pt")
                        for hf in range(2):
                            hsl = slice(hf * 1024, (hf + 1) * 1024)
                            nc.scalar.activation(
                                ptt[:, hsl], spt[:, hsl], AF.Exp,
                                bias=mb[:, t:t + 1], scale=1.0,
                            )
                        for nb in range(S // 512):
                            nsl = slice(nb * 512, (nb + 1) * 512)
                            nc.tensor.matmul(
                                outp[:, nsl], V[:, t, h * 65:h * 65 + 65],
                                ptt[:, nsl], start=(t == 0), stop=(t == NT - 1),
                                skip_group_check=True,
                            )
                    nc.scalar.activation(OT[h][:], outp[:], AF.Copy, bias=0.0)
                    nc.sync.dma_start(out_d.ap()[h * 65:(h + 1) * 65, :], OT[h][:])

    nc.compile()
    return nc


class _Runner:
    """Cached PJRT executable for the SPMD bass kernel.

    Replicates concourse.bass2jax.run_bass_via_pjrt but builds the jitted
    shard_map once (run_bass_via_pjrt re-traces per call) and keeps inputs
    device-resident so repeat executions cost dispatch+execute only.
    """

    def __init__(self, nc):
        import jax
        import concourse.mybir as mybir
        from jax.sharding import Mesh, PartitionSpec, NamedSharding
        from jax.experimental.shard_map import shard_map
        from concourse import bass2jax
        from concourse.bass2jax import _bass_exec_p, install_neuronx_cc_hook

        install_neuronx_cc_hook()
        self.jax = jax
        self.nc = nc
        partition_name = (
            nc.partition_id_tensor.name if nc.partition_id_tensor else None
        )
        in_names, out_names, out_avals, zero_outs = [], [], [], []
        for alloc in nc.m.functions[0].allocations:
            if not isinstance(alloc, mybir.MemoryLocationSet):
                continue
            name = alloc.memorylocations[0].name
            if alloc.kind == "ExternalInput":
                if name != partition_name:
                    in_names.append(name)
            elif alloc.kind == "ExternalOutput":
                out_names.append(name)
                shape = tuple(alloc.tensor_shape)
                dtype = mybir.dt.np(alloc.dtype)
                out_avals.append(jax.core.ShapedArray(shape, dtype))
                zero_outs.append(np.zeros(shape, dtype))
        self.in_names, self.out_names = in_names, out_names
        self.out_avals = out_avals
        n_params, n_outs = len(in_names), len(out_names)
        in_names_all = in_names + out_names
        if partition_name is not None:
            in_names_all.append(partition_name)

        def _body(*args):
            operands = list(args)
            if partition_name is not None:
                operands.append(bass2jax.partition_id_tensor())
            outs = _bass_exec_p.bind(
                *operands,
                out_avals=tuple(out_avals),
                in_names=tuple(in_names_all),
                out_names=tuple(out_names),
                lowering_input_output_aliases=(),
                sim_require_finite=True,
                sim_require_nnan=True,
                nc=nc,
            )
            return tuple(outs)

        devices = jax.devices()[:NCORES]
        mesh = Mesh(np.asarray(devices), ("core",))
        in_specs = (PartitionSpec("core"),) * (n_params + n_outs)
        out_specs = (PartitionSpec("core"),) * n_outs
        # no donation: the zero output-seed buffers stay valid for reuse
        self.sharded = jax.jit(
            shard_map(
                _body, mesh=mesh, in_specs=in_specs, out_specs=out_specs,
                check_rep=False,
            ),
            keep_unused=True,
        )
        self.shard = NamedSharding(mesh, PartitionSpec("core"))
        self.dz = [
            jax.device_put(
                np.zeros((NCORES * z.shape[0], *z.shape[1:]), z.dtype), self.shard
            )
            for z in zero_outs
        ]
        self.dev_in = None

    def stage(self, in_maps):
        """Concat per-core inputs and push to device; cache on self."""
        per_core = [[np.asarray(m[n]) for n in self.in_names] for m in in_maps]
        concat_in = [
            np.concatenate([per_core[c][i] for c in range(NCORES)], axis=0)
            for i in range(len(self.in_names))
        ]
        self.dev_in = [self.jax.device_put(a, self.shard) for a in concat_in]
        self.jax.block_until_ready(self.dev_in)

    def dispatch(self):
        """Async dispatch one kernel execution on staged inputs."""
        return self.sharded(*self.dev_in, *self.dz)

    def run_fetch(self):
        """Execute once and fetch per-core outputs to host."""
        outs = self.dispatch()
        host = [np.asarray(o) for o in outs]
        return [
            {
                n: host[i].reshape(NCORES, *self.out_avals[i].shape)[c]
                for i, n in enumerate(self.out_names)
            }
            for c in range(NCORES)
        ]


def _make_in_maps(x, mask, Wq, bq, Aq, Bq, Wk, bk, Wv, bv, Av, Bv):
    isc = 1.0 / np.sqrt(np.float32(W))
    in_maps = []
    for core in range(NCORES):
        b, g = core // 4, core % 4
        rows = slice(g * WPC, (g + 1) * WPC)
        Wq_s = (Wq[rows] * isc).astype(np.float32)
        bq_s = (bq[rows] * isc).astype(np.float32)
        Bq_s = (Bq[rows] * (isc * LORA_SCALE)).astype(np.float32)
        Wk_s, bk_s = Wk[rows], bk[rows]
        Wv_s, bv_s = Wv[rows], bv[rows]
        Bv_s = (Bv[rows] * LORA_SCALE).astype(np.float32)
        WA = np.concatenate(
            [Wq_s, Wk_s, Aq, np.zeros((16, D), np.float32), Av], axis=0
        )  # [432, 768]
        bias = np.zeros((128, 4), np.float32)
        bias[:, 0] = bq_s[0:128]
        bias[0:64, 1] = bq_s[128:192]
        bias[:, 2] = bk_s[0:128]
        bias[0:64, 3] = bk_s[128:192]
        mb = (-10000.0 * (1.0 - mask[b].astype(np.float32))).reshape(NT, 128).T
        in_maps.append({
            "xT": _bf(np.ascontiguousarray(x[b].T)),
            "WAT": _bf(np.ascontiguousarray(WA.T)),
            "WvT": _bf(np.ascontiguousarray(Wv_s.T)),
            "BqT": _bf(np.ascontiguousarray(Bq_s.T)),
            "BvT": _bf(np.ascontiguousarray(Bv_s.T)),
            "bias_qk": bias,
            "bv_row": _bf(bv_s.reshape(1, WPC)),
            "mb": np.ascontiguousarray(mb),
        })
    return in_maps


def _unshard(results):
    out = np.empty((B, S, D), np.float32)
    for core in range(NCORES):
        b, g = core // 4, core % 4
        ot = results[core]["outT"].reshape(HPC, 65, S)
        for h in range(HPC):
            blk = ot[h, 0:64, :] / ot[h, 64:65, :]
            out[b, :, g * WPC + h * W:(g * WPC) + (h + 1) * W] = blk.T
    return out


def get_runner():
    if "runner" not in _cache:
        _cache["runner"] = _Runner(_build())
    return _cache["runner"]


def kernel(x, mask, Wq, bq, Aq, Bq, Wk, bk, Wv, bv, Av, Bv):
    x, mask = np.asarray(x), np.asarray(mask)
    Wq, bq, Aq, Bq = map(np.asarray, (Wq, bq, Aq, Bq))
    Wk, bk, Wv, bv, Av, Bv = map(np.asarray, (Wk, bk, Wv, bv, Av, Bv))
    r = get_runner()
    r.stage(_make_in_maps(x, mask, Wq, bq, Aq, Bq, Wk, bk, Wv, bv, Av, Bv))
    return _unshard(r.run_fetch())


def _bf(a):
    import jax.numpy as jnp

    return np.asarray(jnp.asarray(np.asarray(a, np.float32), jnp.bfloat16))


# revision 40
# speedup vs baseline: 10466.6934x; 7.5910x over previous
import sys

sys.path.insert(0, "/opt/trn_rl_repo")
import numpy as np

B, S, D, H, R = 2, 2048, 768, 12, 16
LORA_SCALE = 1.0 / R
W = D // H  # 64
HPC = 3  # heads per core
WPC = HPC * W  # 192 output dims per core
NCORES = 8
KP_DEFAULT = 1280  # padded count of unmasked keys (10 x 128)

_cache = {}


def _build(kiter=1, parts="full", kp=KP_DEFAULT, nk2=None, depth=6,
           spbufs=3, opbufs=1, mmw=512):
    from contextlib import nullcontext

    import concourse.bacc as bacc
    import concourse.mybir as mybir
    import concourse.tile as tile

    f32 = mybir.dt.float32
    bf16 = mybir.dt.bfloat16
    AF = mybir.ActivationFunctionType
    NK = kp // 128  # key chunks

    nc = bacc.Bacc("TRN2", target_bir_lowering=False, debug=False)
    xT_d = nc.dram_tensor("xT", [D, S], bf16, kind="ExternalInput")
    xKV_d = nc.dram_tensor("xKV", [D, kp], bf16, kind="ExternalInput")
    WqA_d = nc.dram_tensor("WqA", [D, 208], bf16, kind="ExternalInput")
    WkA_d = nc.dram_tensor("WkA", [D, 208], bf16, kind="ExternalInput")
    Wv195_d = nc.dram_tensor("Wv195", [D, 195], bf16, kind="ExternalInput")
    BqT_d = nc.dram_tensor("BqT", [R, WPC], bf16, kind="ExternalInput")
    Bv195_d = nc.dram_tensor("Bv195", [17, 195], bf16, kind="ExternalInput")
    biasq_d = nc.dram_tensor("biasq", [128, 2], f32, kind="ExternalInput")
    biask_d = nc.dram_tensor("biask", [128, 2], f32, kind="ExternalInput")
    mbk_d = nc.dram_tensor("mbk", [128, NK], f32, kind="ExternalInput")
    out_d = nc.dram_tensor("outT", [HPC * 65, S], f32, kind="ExternalOutput")

    with tile.TileContext(nc) as tc:
        with (
            tc.For_i(0, kiter) if kiter > 1 else nullcontext(),
            tc.tile_pool(name="cst", bufs=1) as cst,
        ):
            xT = cst.tile([128, 6, S], bf16, name="xT")
            xKV = cst.tile([128, 6, kp], bf16, name="xKV")
            WqA = cst.tile([128, 6, 208], bf16, name="WqA")
            WkA = cst.tile([128, 6, 208], bf16, name="WkA")
            Wv195 = cst.tile([128, 6, 195], bf16, name="Wv195")
            BqT = cst.tile([R, WPC], bf16, name="BqT")
            Bv195 = cst.tile([17, 195], bf16, name="Bv195")
            biasq = cst.tile([128, 2], f32, name="biasq")
            biask = cst.tile([128, 2], f32, name="biask")
            mbk = cst.tile([128, NK], f32, name="mbk")
            QT = cst.tile([128, 2, S], bf16, name="QT")
            KT = cst.tile([128, 2, kp], bf16, name="KT")
            uq = cst.tile([R, S], bf16, name="uq")
            uvo = cst.tile([17, kp], bf16, name="uvo")  # 16 uv rows + ones
            V = cst.tile([128, NK, 195], bf16, name="V")
            OT = [cst.tile([65, S], f32, name=f"ot{h}") for h in range(HPC)]

            # No all-engine barrier: Tile orders each DMA before its first
            # reader.  xKV/WkA load first so the k stream computes while the
            # larger xT transfer is still in flight.
            nc.vector.memset(uvo[:], 1.0)  # row 16 stays 1.0; rows 0:16
            # are overwritten by the uv drain before any read
            nc.sync.dma_start(WkA[:], WkA_d.ap().rearrange("(c p) m -> p c m", p=128))
            nc.sync.dma_start(xKV[:], xKV_d.ap().rearrange("(c p) s -> p c s", p=128))
            nc.sync.dma_start(Wv195[:], Wv195_d.ap().rearrange("(c p) m -> p c m", p=128))
            nc.sync.dma_start(WqA[:], WqA_d.ap().rearrange("(c p) m -> p c m", p=128))
            nc.sync.dma_start(xT[:], xT_d.ap().rearrange("(c p) s -> p c s", p=128))
            nc.gpsimd.dma_start(BqT[:], BqT_d.ap())
            nc.gpsimd.dma_start(Bv195[:], Bv195_d.ap())
            nc.gpsimd.dma_start(biasq[:], biasq_d.ap())
            nc.gpsimd.dma_start(biask[:], biask_d.ap())
            nc.gpsimd.dma_start(mbk[:], mbk_d.ap())

            # ---- phase 1: projections ----
            # k stream first (xKV is the smaller DMA): chunk1 = [k 128:192 |
            # Av] (80 rows), chunk0 = k 0:128.  No LoRA on k; uv falls out of
            # chunk1 rows 64:80.  Per-slice drains release PSUM banks early.
            if parts in ("full", "p1"):
                kslices = [slice(i, min(i + 512, kp)) for i in range(0, kp, 512)]
                with (
                    tc.tile_pool(name="pk0", bufs=1, space="PSUM") as pk0p,
                    tc.tile_pool(name="pk1", bufs=1, space="PSUM") as pk1p,
                ):
                    pk1 = pk1p.tile([80, kp], f32, name="pk1")
                    pk0 = pk0p.tile([128, kp], f32, name="pk0")
                    for ssl in kslices:
                        for c in range(6):
                            nc.tensor.matmul(
                                pk1[:, ssl], WkA[:, c, 128:208], xKV[:, c, ssl],
                                start=(c == 0), stop=(c == 5),
                            )
                        nc.vector.tensor_copy(uvo[0:16, ssl], pk1[64:80, ssl])
                        nc.vector.tensor_scalar_add(
                            KT[0:64, 1, ssl], pk1[0:64, ssl], biask[0:64, 1:2]
                        )
                    for ssl in kslices:
                        for c in range(6):
                            nc.tensor.matmul(
                                pk0[:, ssl], WkA[:, c, 0:128], xKV[:, c, ssl],
                                start=(c == 0), stop=(c == 5), skip_group_check=True,
                            )
                        nc.vector.tensor_scalar_add(
                            KT[:, 0, ssl], pk0[:, ssl], biask[:, 0:1]
                        )

                # v stream: per 128-key chunk, 195-col layout (64 v dims + ones
                # col per head).  LoRA-B + bias + ones ride in via Bv195/uvo.
                # The drain scales by mbk (1 real / 0 padded), zeroing padded
                # keys' V rows AND their ones column -- they then contribute
                # exactly 0 to both the numerator and the softmax denominator,
                # so phase 2 needs no mask bias at all.
                with (
                    tc.tile_pool(name="vpa", bufs=1, space="PSUM") as vpa,
                    tc.tile_pool(name="vpb", bufs=1, space="PSUM") as vpb,
                ):
                    for t in range(NK):
                        tsl = slice(t * 128, (t + 1) * 128)
                        p = (vpa if t % 2 == 0 else vpb).tile(
                            [128, 195], f32, name="vpt"
                        )
                        nc.tensor.matmul(p[:], uvo[:, tsl], Bv195[:], start=True,
                                         stop=False, skip_group_check=True)
                        for c in range(6):
                            nc.tensor.matmul(
                                p[:], xKV[:, c, tsl], Wv195[:, c, :],
                                start=False, stop=(c == 5), skip_group_check=True,
                            )
                        nc.vector.tensor_scalar_mul(
                            V[:, t, :], p[:], mbk[:, t:t + 1]
                        )

                # q stream: chunk1 = [q 128:192 | Aq] (80 rows), chunk0 =
                # q 0:128.  Whole-S PSUM residency: 4 + 4 banks.  uq falls
                # out of chunk1 rows 64:80; LoRA-B mms accumulate after.
                with (
                    tc.tile_pool(name="pq0", bufs=1, space="PSUM") as pq0p,
                    tc.tile_pool(name="pq1", bufs=1, space="PSUM") as pq1p,
                ):
                    pq1 = pq1p.tile([80, S], f32, name="pq1")
                    pq0 = pq0p.tile([128, S], f32, name="pq0")
                    for sb in range(S // 512):
                        ssl = slice(sb * 512, (sb + 1) * 512)
                        for c in range(6):
                            nc.tensor.matmul(
                                pq1[:, ssl], WqA[:, c, 128:208], xT[:, c, ssl],
                                start=(c == 0), stop=(c == 5),
                            )
                        nc.vector.tensor_copy(uq[:, ssl], pq1[64:80, ssl])
                    for sb in range(S // 512):
                        ssl = slice(sb * 512, (sb + 1) * 512)
                        for c in range(6):
                            nc.tensor.matmul(
                                pq0[:, ssl], WqA[:, c, 0:128], xT[:, c, ssl],
                                start=(c == 0), stop=False, skip_group_check=True,
                            )
                        nc.tensor.matmul(
                            pq0[:, ssl], BqT[:, 0:128], uq[:, ssl],
                            start=False, stop=True, skip_group_check=True,
                        )
                        nc.vector.tensor_scalar_add(
                            QT[:, 0, ssl], pq0[:, ssl], biasq[:, 0:1]
                        )
                    for sb in range(S // 512):
                        ssl = slice(sb * 512, (sb + 1) * 512)
                        nc.tensor.matmul(
                            pq1[0:64, ssl], BqT[:, 128:192], uq[:, ssl],
                            start=False, stop=True, skip_group_check=True,
                        )
                        nc.vector.tensor_scalar_add(
                            QT[0:64, 1, ssl], pq1[0:64, ssl], biasq[0:64, 1:2]
                        )

            # ---- phase 2: attention over compacted keys ----
            # One flat stream of (head, q-half, key-chunk) tiles with the AV
            # matmuls emitted 2 chunks behind the scores matmuls, so the
            # scores->exp->AV semaphore latency never stalls PE and the
            # pipeline doesn't drain at head/q-half boundaries.
            QH = 1024
            DEPTH = depth
            qk_src = [(0, 0), (0, 64), (1, 0)]
            with (
                tc.tile_pool(name="sp", bufs=spbufs, space="PSUM") as sp,
                tc.tile_pool(name="op", bufs=opbufs, space="PSUM") as op,
                tc.tile_pool(name="pt", bufs=DEPTH + 2) as ptp,
            ):
                outps = {}
                pend = []

                NKE = nk2 if nk2 else NK

                def av(h, qh, t, ptt):
                    v_ap = V[:, :, h * 65:h * 65 + 65]
                    outp = outps[(h, qh)]
                    for nb in range(QH // mmw):
                        nsl = slice(nb * mmw, (nb + 1) * mmw)
                        nc.tensor.matmul(
                            outp[:, nsl], v_ap[:, t, :], ptt[:, nsl],
                            start=(t == 0), stop=(t == NKE - 1),
                            skip_group_check=True,
                        )
                    if t == NKE - 1:
                        qsl = slice(qh * QH, (qh + 1) * QH)
                        nc.vector.tensor_copy(OT[h][:, qsl], outp[:])
                        nc.sync.dma_start(
                            out_d.ap()[h * 65:(h + 1) * 65, qsl],
                            OT[h][:, qsl],
                        )
                        del outps[(h, qh)]

                nheads = HPC if parts in ("full", "p2") else 0
                for h in range(nheads):
                    ci, pb = qk_src[h]
                    q_ap = QT[pb:pb + 64, ci, :]
                    for qh in range(S // QH):
                        outps[(h, qh)] = op.tile([65, QH], f32, name="op")
                        for t in range(NKE):
                            # delayed AV first: its act dependency is long
                            # satisfied, and it clears the WAR hazard on the
                            # score buffer the next matmul is about to reuse
                            if len(pend) > DEPTH - 1:
                                av(*pend.pop(0))
                            spt = sp.tile([128, QH], f32, name="sp")
                            for nb in range(QH // mmw):
                                gsl = slice(qh * QH + nb * mmw,
                                            qh * QH + (nb + 1) * mmw)
                                nc.tensor.matmul(
                                    spt[:, nb * mmw:(nb + 1) * mmw],
                                    KT[pb:pb + 64, ci, t * 128:(t + 1) * 128],
                                    q_ap[:, gsl], start=True, stop=True,
                                )
                            ptt = ptp.tile([128, QH], bf16, name="pt")
                            nc.scalar.activation(
                                ptt[:], spt[:], AF.Exp, bias=0.0, scale=1.0,
                            )
                            pend.append((h, qh, t, ptt))
                while pend:
                    av(*pend.pop(0))

    nc.compile()
    return nc


class _Runner:
    """Cached PJRT executable for the SPMD bass kernel.

    Replicates concourse.bass2jax.run_bass_via_pjrt but builds the jitted
    shard_map once (run_bass_via_pjrt re-traces per call) and keeps inputs
    device-resident so repeat executions cost dispatch+execute only.
    """

    def __init__(self, nc):
        import jax
        import concourse.mybir as mybir
        from jax.sharding import Mesh, PartitionSpec, NamedSharding
        from jax.experimental.shard_map import shard_map
        from concourse import bass2jax
        from concourse.bass2jax import _bass_exec_p, install_neuronx_cc_hook

        install_neuronx_cc_hook()
        self.jax = jax
        self.nc = nc
        partition_name = (
            nc.partition_id_tensor.name if nc.partition_id_tensor else None
        )
        in_names, out_names, out_avals, zero_outs = [], [], [], []
        for alloc in nc.m.functions[0].allocations:
            if not isinstance(alloc, mybir.MemoryLocationSet):
                continue
            name = alloc.memorylocations[0].name
            if alloc.kind == "ExternalInput":
                if name != partition_name:
                    in_names.append(name)
            elif alloc.kind == "ExternalOutput":
                out_names.append(name)
                shape = tuple(alloc.tensor_shape)
                dtype = mybir.dt.np(alloc.dtype)
                out_avals.append(jax.core.ShapedArray(shape, dtype))
                zero_outs.append(np.zeros(shape, dtype))
        self.in_names, self.out_names = in_names, out_names
        self.out_avals = out_avals
        n_params, n_outs = len(in_names), len(out_names)
        in_names_all = in_names + out_names
        if partition_name is not None:
            in_names_all.append(partition_name)

        def _body(*args):
            operands = list(args)
            if partition_name is not None:
                operands.append(bass2jax.partition_id_tensor())
            outs = _bass_exec_p.bind(
                *operands,
                out_avals=tuple(out_avals),
                in_names=tuple(in_names_all),
                out_names=tuple(out_names),
                lowering_input_output_aliases=(),
                sim_require_finite=True,
                sim_require_nnan=True,
                nc=nc,
            )
            return tuple(outs)

        devices = jax.devices()[:NCORES]
        mesh = Mesh(np.asarray(devices), ("core",))
        in_specs = (PartitionSpec("core"),) * (n_params + n_outs)
        out_specs = (PartitionSpec("core"),) * n_outs
        # no donation: the zero output-seed buffers stay valid for reuse
        self.sharded = jax.jit(
            shard_map(
                _body, mesh=mesh, in_specs=in_specs, out_specs=out_specs,
                check_rep=False,
            ),
            keep_unused=True,
        )
        self.shard = NamedSharding(mesh, PartitionSpec("core"))
        self.dz = [
            jax.device_put(
                np.zeros((NCORES * z.shape[0], *z.shape[1:]), z.dtype), self.shard
            )
            for z in zero_outs
        ]
        self.dev_in = None

    def stage(self, in_maps):
        """Concat per-core inputs and push to device; cache on self."""
        per_core = [[np.asarray(m[n]) for n in self.in_names] for m in in_maps]
        concat_in = [
            np.concatenate([per_core[c][i] for c in range(NCORES)], axis=0)
            for i in range(len(self.in_names))
        ]
        self.dev_in = [self.jax.device_put(a, self.shard) for a in concat_in]
        self.jax.block_until_ready(self.dev_in)

    def dispatch(self):
        """Async dispatch one kernel execution on staged inputs."""
        return self.sharded(*self.dev_in, *self.dz)

    def run_fetch(self):
        """Execute once and fetch per-core outputs to host."""
        outs = self.dispatch()
        host = [np.asarray(o) for o in outs]
        return [
            {
                n: host[i].reshape(NCORES, *self.out_avals[i].shape)[c]
                for i, n in enumerate(self.out_names)
            }
            for c in range(NCORES)
        ]


def _pick_kp(mask):
    """Smallest multiple of 128 covering the max unmasked-key count."""
    nk = int(max(mask[b].sum() for b in range(B)))
    return min(max(-(-nk // 128) * 128, 256), S)


def _make_in_maps(x, mask, Wq, bq, Aq, Bq, Wk, bk, Wv, bv, Av, Bv, kp):
    isc = 1.0 / np.sqrt(np.float32(W))
    NK = kp // 128
    in_maps = []
    for core in range(NCORES):
        b, g = core // 4, core % 4
        rows = slice(g * WPC, (g + 1) * WPC)
        Wq_s = (Wq[rows] * isc).astype(np.float32)
        bq_s = (bq[rows] * isc).astype(np.float32)
        Bq_s = (Bq[rows] * (isc * LORA_SCALE)).astype(np.float32)
        Wk_s, bk_s = Wk[rows], bk[rows]
        Wv_s, bv_s = Wv[rows], bv[rows]
        Bv_s = (Bv[rows] * LORA_SCALE).astype(np.float32)

        # compacted keys for this batch
        idx = np.nonzero(mask[b])[0]
        nk = len(idx)
        xKV = np.zeros((D, kp), np.float32)
        xKV[:, :nk] = x[b][idx].T

        # q stream weights: cols 0:128 = q chunk0, 128:192 = q chunk1,
        # 192:208 = Aq
        WqA = np.zeros((D, 208), np.float32)
        WqA[:, 0:192] = Wq_s.T
        WqA[:, 192:208] = Aq.T
        # k stream weights: cols 0:192 = k, 192:208 = Av
        WkA = np.zeros((D, 208), np.float32)
        WkA[:, 0:192] = Wk_s.T
        WkA[:, 192:208] = Av.T
        # v in 195-col layout: per head h, cols h*65:h*65+64 = Wv dims,
        # col h*65+64 = 0 (ones col, produced via Bv195 row 16)
        Wv195 = np.zeros((D, 195), np.float32)
        Bv195 = np.zeros((17, 195), np.float32)
        for hh in range(HPC):
            csl = slice(hh * 65, hh * 65 + 64)
            wsl = slice(hh * 64, (hh + 1) * 64)
            Wv195[:, csl] = Wv_s.T[:, wsl]
            Bv195[0:16, csl] = Bv_s.T[:, wsl]
            Bv195[16, csl] = bv_s[wsl]
            Bv195[16, hh * 65 + 64] = 1.0

        biasq = np.zeros((128, 2), np.float32)
        biasq[:, 0] = bq_s[0:128]
        biasq[0:64, 1] = bq_s[128:192]
        biask = np.zeros((128, 2), np.float32)
        biask[:, 0] = bk_s[0:128]
        biask[0:64, 1] = bk_s[128:192]

        # multiplicative key mask: 1 for real keys, 0 for padded slots
        # (applied to V rows, so padded keys add 0 to numerator+denominator)
        mbk = np.zeros((kp,), np.float32)
        mbk[:nk] = 1.0
        mbk = np.ascontiguousarray(mbk.reshape(NK, 128).T)

        in_maps.append({
            "xT": _bf(np.ascontiguousarray(x[b].T)),
            "xKV": _bf(xKV),
            "WqA": _bf(WqA),
            "WkA": _bf(WkA),
            "Wv195": _bf(Wv195),
            "BqT": _bf(np.ascontiguousarray(Bq_s.T)),
            "Bv195": _bf(Bv195),
            "biasq": biasq,
            "biask": biask,
            "mbk": mbk,
        })
    return in_maps


def _unshard(results):
    out = np.empty((B, S, D), np.float32)
    for core in range(NCORES):
        b, g = core // 4, core % 4
        ot = results[core]["outT"].reshape(HPC, 65, S)
        for h in range(HPC):
            blk = ot[h, 0:64, :] / ot[h, 64:65, :]
            out[b, :, g * WPC + h * W:(g * WPC) + (h + 1) * W] = blk.T
    return out


def get_runner(kiter=1, parts="full", kp=KP_DEFAULT, nk2=None, depth=6,
               spbufs=3, opbufs=1, mmw=512):
    key = ("runner", kiter, parts, kp, nk2, depth, spbufs, opbufs, mmw)
    if key not in _cache:
        _cache[key] = _Runner(
            _build(kiter, parts, kp, nk2, depth, spbufs, opbufs, mmw)
        )
    return _cache[key]


def kernel(x, mask, Wq, bq, Aq, Bq, Wk, bk, Wv, bv, Av, Bv):
    x, mask = np.asarray(x), np.asarray(mask)
    Wq, bq, Aq, Bq = map(np.asarray, (Wq, bq, Aq, Bq))
    Wk, bk, Wv, bv, Av, Bv = map(np.asarray, (Wk, bk, Wv, bv, Av, Bv))
    kp = _pick_kp(mask)
    r = get_runner(kp=kp)
    r.stage(_make_in_maps(x, mask, Wq, bq, Aq, Bq, Wk, bk, Wv, bv, Av, Bv, kp))
    return _unshard(r.run_fetch())


def _bf(a):
    import jax.numpy as jnp

    return np.asarray(jnp.asarray(np.asarray(a, np.float32), jnp.bfloat16))
